# revision 20
# baseline (speedup 1.0000x reference)
"""Causal self-attention Bass kernel for 8x Trainium2 NeuronCores.

Problem: B=8, T=1024, D=1024, H=16 heads (head_dim 64), fp32.
Sharding: data parallel over batch -- each of the 8 cores handles one
batch element with replicated weights; outputs are stacked on the host.

Per-core dataflow (all matmuls on PE in bf16 with fp32 PSUM accumulate;
weights are cast to bf16 on the host):
  1. x [T,D] is host-cast to bf16 (halves the startup DMA) and
     transposed on PE (128x128 blocks, 1 cycle/row at bf16) to xT [D,T];
     crossbar-transpose DMAs were tried and are ~8x too slow.
  2. v = x @ w_qkv[:,2048:] in natural layout with the bias added on DVE
     during the PSUM->SBUF cast against a stride-0-DMA bias tile; per
     head a ones column is interleaved at slot 0 so the AV matmul
     accumulates the softmax denominator at PSUM partition 0.
     qkT = (w_qkv[:, :2048])^T @ x^T kept transposed [2048,T]; q/k bias
     folded into the PSUM->SBUF cast as a per-partition DVE add.
     qk f-tiles are produced in chunk order (q03, k03, q47, k47) with
     head attention interleaved between the chunks so ACT/DVE softmax
     work overlaps the dense qk matmul stream on PE.
  3. Per head h: scoresT[tk, tq 0:1024] for both tq halves live in one
     two-bank PSUM tile, exp'd by a single bank-crossing ACT op per tk
     tile (scale=1/8 folded; no max-subtraction -- scores are O(1) so
     exp cannot overflow).  Causal handling computes only the exact
     unmasked column window (128-granular) plus one [128,128] triangular
     mask multiply per diagonal block on DVE.  o_aug[0:65, tq] +=
     v_aug^T @ P accumulates both halves in one two-bank PSUM tile.
     QK(i+1) is emitted before AV(i) so the exp chain never stalls PE.
  4. Normalization: one DVE reciprocal_approx_fast per head on the
     [1,1024] denominator row at PSUM partition 0, broadcast to 64
     partitions through a DRAM bounce with a stride-0 read (idle DMA
     bandwidth only), multiplied into the attention tile on DVE --
     deferred by one head so PE never waits on the chain.
  5. y = attn^T' @ w_proj with the bias added on DVE during the
     PSUM->SBUF copy, streamed back to DRAM.
"""

import numpy as np
from contextlib import ExitStack

import concourse.bass as bass
import concourse.bacc as bacc
import concourse.tile as tile
import concourse.mybir as mybir
from concourse import bass_utils

F32 = mybir.dt.float32
BF16 = mybir.dt.bfloat16
AF = mybir.ActivationFunctionType
OP = mybir.AluOpType

B, T, D, H, HD = 8, 1024, 1024, 16, 64
P = 128
N_CORES = 8

TRACE = False

_CACHE = {}
LAST_RESULT = {}


def _build_tile_kernel(nc, aps):
    x, wq, bq, wp, bp, tri, bqv, out = (
        aps["x"], aps["w_qkv"], aps["b_qkv"], aps["w_proj"], aps["b_proj"],
        aps["tri"], aps["bqv"], aps["out"],
    )

    with tile.TileContext(nc) as tc, ExitStack() as ctx:
        consts = ctx.enter_context(tc.tile_pool(name="consts", bufs=1))
        qk_pool = ctx.enter_context(tc.tile_pool(name="qk_pool", bufs=16))
        xt_pool = ctx.enter_context(tc.tile_pool(name="xt_pool", bufs=8))
        v_pool = ctx.enter_context(tc.tile_pool(name="v_pool", bufs=8))
        w_pool = ctx.enter_context(tc.tile_pool(name="w_pool", bufs=12))
        xn_pool = ctx.enter_context(tc.tile_pool(name="xn_pool", bufs=8))
        at_pool = ctx.enter_context(tc.tile_pool(name="at_pool", bufs=8))
        p_pool = ctx.enter_context(tc.tile_pool(name="p_pool", bufs=6))
        den_pool = ctx.enter_context(tc.tile_pool(name="den_pool", bufs=2))
        rb_pool = ctx.enter_context(tc.tile_pool(name="rb_pool", bufs=3))
        dr_pool = ctx.enter_context(
            tc.tile_pool(name="dr_pool", bufs=4, space="DRAM"))
        y_pool = ctx.enter_context(tc.tile_pool(name="y_pool", bufs=2))
        ps = ctx.enter_context(tc.tile_pool(name="ps", bufs=2, space="PSUM"))
        ops = ctx.enter_context(tc.tile_pool(name="ops", bufs=2, space="PSUM"))

        # ---- phase 1a: x -> xT (PE transpose of 128x128 blocks) --------
        # x and the identity go into the DMA queues FIRST -- everything at
        # startup waits on xT, and the 1MB of bias-broadcast DMAs would
        # otherwise delay it by ~10us.
        id_sb = consts.tile([P, P], BF16)
        nc.sync.dma_start(out=id_sb, in_=aps["ident"])
        xns = []
        for ti in range(8):
            xn = xn_pool.tile([P, D], BF16, name="xn", tag="xn")
            nc.sync.dma_start(out=xn, in_=x[ti * P:(ti + 1) * P, :])
            xns.append(xn)

        # ---- constants -------------------------------------------------
        tri_sb = consts.tile([P, P], BF16)
        nc.sync.dma_start(out=tri_sb, in_=tri)
        bcol_sb = consts.tile([P, 16], F32)  # b_qkv[0:2048] as per-partition cols
        nc.sync.dma_start(out=bcol_sb, in_=bq[0:2048].rearrange("(f p) -> p f", p=P))
        # bias rows replicated to all 128 partitions by stride-0 DMA (bf16
        # halves the broadcast bytes; the biases only feed bf16-adjacent
        # adds)
        bv_b = consts.tile([P, D], BF16)
        nc.sync.dma_start(out=bv_b, in_=aps["bqv16"].to_broadcast((P, D)))
        bp_b = consts.tile([P, D], BF16)
        nc.sync.dma_start(out=bp_b, in_=aps["bp16"].to_broadcast((P, D)))
        xt_tiles = [
            xt_pool.tile([P, T], BF16, name="xt", tag="xt") for _ in range(8)
        ]
        for jj in range(2):
            for r in range(4):
                ka, kb = r, r + 4
                pst = ps.tile([P, 1024], BF16, name="pst", tag="ps")
                for tt in range(4):
                    xn = xns[jj * 4 + tt]
                    nc.tensor.transpose(
                        pst[:, tt * P:(tt + 1) * P],
                        xn[:, ka * P:(ka + 1) * P], id_sb)
                    nc.tensor.transpose(
                        pst[:, 512 + tt * P:512 + (tt + 1) * P],
                        xn[:, kb * P:(kb + 1) * P], id_sb)
                nc.vector.tensor_copy(
                    xt_tiles[ka][:, jj * 512:(jj + 1) * 512], pst[:, 0:512])
                nc.vector.tensor_copy(
                    xt_tiles[kb][:, jj * 512:(jj + 1) * 512], pst[:, 512:1024])

        # ---- phase 1v: v natural layout ---------------------------------
        # v_tiles[m] is [128, 16*128] bf16: per head [ones | 63 pad | 64 v]
        # so the AV matmul puts the softmax denominator at PSUM partition 0
        # (readable by reciprocal_approx_fast) and o rows at partitions
        # 64..127 (64-partition DVE accesses must start at 0 or 64).
        v_tiles = []
        for m in range(8):
            vt = v_pool.tile([P, 16 * 128], BF16, name="vt", tag="v")
            rr0 = vt.rearrange("p (h c) -> p h c", c=128)
            nc.vector.memset(rr0[:, :, 0:1], 1.0)
            nc.vector.memset(rr0[:, :, 1:64], 0.0)
            v_tiles.append(vt)
        vw_tiles = {}
        for k in range(8):
            wt = w_pool.tile([P, 1024], BF16, name="wt", tag="w")
            nc.sync.dma_start(
                out=wt, in_=wq[k * P:(k + 1) * P, 2048:3072]
            )
            vw_tiles[k] = wt
        for m in range(8):
            acc = ps.tile([P, 1024], F32, name="acc", tag="ps")
            for k in range(8):
                xsl = xt_tiles[k][:, m * P:(m + 1) * P]
                nc.tensor.matmul(acc[:, 0:512], xsl, vw_tiles[k][:, 0:512],
                                 start=(k == 0), stop=(k == 7))
                nc.tensor.matmul(acc[:, 512:1024], xsl, vw_tiles[k][:, 512:1024],
                                 start=(k == 0), stop=(k == 7))
            rr = v_tiles[m].rearrange("p (h c) -> p h c", c=128)
            nc.vector.tensor_tensor(
                rr[:, :, 64:128], acc.rearrange("p (h c) -> p h c", c=64),
                bv_b.rearrange("p (h c) -> p h c", c=64), op=OP.add,
            )

        # ---- phase 1q + 2: qkT production interleaved with attention ---
        qk_tiles = {}  # f-tile index 0..15 -> [128, 1024] bf16

        def emit_qk_tile(f):
            # all 8 K-slices of w for this f-tile in one strided DMA
            wt = w_pool.tile([P, 8, P], BF16, name="wt", tag="w")
            nc.sync.dma_start(
                out=wt,
                in_=wq[:, f * P:(f + 1) * P].rearrange(
                    "(k p) f -> p k f", p=P),
            )
            qk_t = qk_pool.tile([P, T], BF16, name="qk_t", tag="qk")
            qk_tiles[f] = qk_t
            acc = ps.tile([P, 1024], F32, name="acc", tag="ps")
            for k in range(8):
                nc.tensor.matmul(acc[:, 0:512], wt[:, k, :],
                                 xt_tiles[k][:, 0:512],
                                 start=(k == 0), stop=(k == 7))
            for k in range(8):
                nc.tensor.matmul(acc[:, 512:1024], wt[:, k, :],
                                 xt_tiles[k][:, 512:1024],
                                 start=(k == 0), stop=(k == 7))
            nc.vector.tensor_scalar_add(qk_t, acc, bcol_sb[:, f:f + 1])

        att_tiles = {}  # hp -> [128, 1024] bf16 (tq halves side by side)
        for hp in range(8):
            att_tiles[hp] = at_pool.tile([P, T], BF16, name="at", tag="at")

        # deferred normalization: PE must never wait on the DVE recip ->
        # DMA bounce -> DVE mult chain, so each chain starts as soon as its
        # tq half finishes accumulating and the multiply lands one head
        # later (pn_early: first halves; pn_late: second halves).
        pn_early = []
        pn_late = []

        def start_norm(o_ps, att_slice, lo, hi, queue):
            dinv = den_pool.tile([1, hi - lo], F32, name="dinv", tag="dinv")
            nc.vector.reciprocal_approx_fast(dinv, o_ps[0:1, lo:hi])
            dd = dr_pool.tile([1, hi - lo], F32, name="dd", tag="dd")
            nc.sync.dma_start(out=dd, in_=dinv)
            rb_sb = rb_pool.tile([64, hi - lo], F32, name="rb", tag="rb")
            nc.sync.dma_start(out=rb_sb, in_=dd.to_broadcast((64, hi - lo)))
            queue.append((rb_sb, o_ps, lo, hi, att_slice))

        def flush_norm(queue):
            while queue:
                rb_sb, o_ps, lo, hi, att_slice = queue.pop(0)
                nc.vector.tensor_tensor(
                    att_slice[:, lo:hi], o_ps[64:128, lo:hi], rb_sb,
                    op=OP.mult)

        def emit_head(h):
            """Attention for head h, both tq halves fused: scores tile i
            covers tq cols [128i..512) (first half, causal window) and
            [512..1024) (second half) in one 2-bank PSUM tile with a
            single bank-crossing exp."""
            fq = h // 2
            po = (h % 2) * 64
            qT = qk_tiles[fq][po:po + 64, :]
            kTt = qk_tiles[8 + fq]
            o_ps = ops.tile([P, 1024], F32, name="o_ps", tag="ops")
            att_slice = att_tiles[h // 2][(h % 2) * 64:(h % 2) * 64 + 64, :]
            pending = None

            def emit_av(pend, stop):
                pi, pp = pend
                va = v_tiles[pi].rearrange("p (h c) -> p h c", c=128)[:, h, :]
                if pi < 4:  # first-half window [128*pi .. 512)
                    nc.tensor.matmul(
                        o_ps[:, P * pi:512], va, pp[:, P * pi:512],
                        start=(pi == 0), stop=(pi == 3),
                        skip_group_check=True,
                    )
                    nc.tensor.matmul(
                        o_ps[:, 512:1024], va, pp[:, 512:1024],
                        start=(pi == 0), stop=stop,
                        skip_group_check=True,
                    )
                else:  # second half only, window [512+128m .. 1024)
                    ws = 512 + P * (pi - 4)
                    nc.tensor.matmul(
                        o_ps[:, ws:], va, pp[:, ws:],
                        start=False, stop=stop,
                        skip_group_check=True,
                    )

            for i in range(8):
                kT = kTt[po:po + 64, i * P:(i + 1) * P]
                s_ps = ps.tile([P, 1024], F32, name="s_ps", tag="ps")
                p_sb = p_pool.tile([P, 1024], BF16, name="p_sb", tag="p")
                if i < 4:
                    ws = P * i
                    nc.tensor.matmul(s_ps[:, ws:512], kT, qT[:, ws:512],
                                     start=True, stop=True)
                    nc.tensor.matmul(s_ps[:, 512:1024], kT, qT[:, 512:1024],
                                     start=True, stop=True)
                    nc.scalar.activation(
                        p_sb[:, ws:], s_ps[:, ws:], AF.Exp, scale=0.125)
                else:
                    ws = 512 + P * (i - 4)
                    nc.tensor.matmul(s_ps[:, ws:], kT, qT[:, ws:],
                                     start=True, stop=True)
                    nc.scalar.activation(
                        p_sb[:, ws:], s_ps[:, ws:], AF.Exp, scale=0.125)
                nc.vector.tensor_tensor(
                    p_sb[:, ws:ws + P], p_sb[:, ws:ws + P],
                    tri_sb, op=OP.mult)
                if i == 2:
                    flush_norm(pn_early)
                if pending is not None:
                    emit_av(pending, stop=False)
                pending = (i, p_sb)
            emit_av(pending, stop=True)
            start_norm(o_ps, att_slice, 0, 1024, pn_early)

        # Head pair (2c, 2c+1) needs q f-tile c and k f-tile 8+c.  Emit
        # each f-tile production (a dense 8.2k-cycle PE chunk) right before
        # the head pair that consumes it, so the ACT/DVE softmax work of
        # every pair overlaps a dense qk matmul stream on PE.
        emit_qk_tile(0)
        emit_qk_tile(8)
        emit_qk_tile(1)
        emit_qk_tile(9)
        emit_qk_tile(2)
        for c in range(8):
            emit_head(2 * c)
            if c < 6:
                emit_qk_tile(10 + c)
            emit_head(2 * c + 1)
            if c < 5:
                emit_qk_tile(c + 3)
        flush_norm(pn_early)

        # ---- phase 3: projection ---------------------------------------
        wp_tiles = {}
        for c in range(8):
            wpt = w_pool.tile([P, 1024], BF16, name="wpt", tag="w")
            nc.sync.dma_start(out=wpt, in_=wp[c * P:(c + 1) * P, :])
            wp_tiles[c] = wpt
        for j in range(2):
            for mi in range(4):
                mrow = 4 * j + mi
                y_ps = ps.tile([P, 1024], F32, name="y_ps", tag="ps")
                for c in range(8):
                    asl = att_tiles[c][:, j * 512 + mi * P:j * 512 + (mi + 1) * P]
                    nc.tensor.matmul(y_ps[:, 0:512], asl, wp_tiles[c][:, 0:512],
                                     start=(c == 0), stop=(c == 7))
                    nc.tensor.matmul(y_ps[:, 512:1024], asl,
                                     wp_tiles[c][:, 512:1024],
                                     start=(c == 0), stop=(c == 7))
                y_sb = y_pool.tile([P, 1024], F32, name="y_sb", tag="y")
                nc.vector.tensor_tensor(y_sb, y_ps, bp_b, op=OP.add)
                nc.sync.dma_start(
                    out=out[mrow * P:(mrow + 1) * P, :], in_=y_sb)


def _pin_act_table(arch):
    """Force every ACT func we use into one table so walrus never emits
    mid-kernel ACT_TABLE_LOADs (each is ~1.3us on the ScalarE stream).
    The cached dict is mutated in place, preserving set ids/order."""
    import concourse.hw_specs as hw_specs
    tabs = hw_specs.get_activation_tables(arch)
    keep = "natural_log_exp_and_others"
    if keep not in tabs:
        return
    need = tabs[keep] & {AF.Exp, AF.Ln, AF.Copy, AF.Identity}
    for name, fns in tabs.items():
        if name != keep:
            fns -= need


def _get_nc():
    if "nc" in _CACHE:
        return _CACHE["nc"]
    nc = bacc.Bacc("TRN2", target_bir_lowering=False, debug=False,
                   num_devices=N_CORES)
    _pin_act_table(nc.m.arch)
    aps = {
        "x": nc.dram_tensor("x", [T, D], BF16, kind="ExternalInput").ap(),
        "w_qkv": nc.dram_tensor("w_qkv", [D, 3 * D], BF16, kind="ExternalInput").ap(),
        "b_qkv": nc.dram_tensor("b_qkv", [3 * D], F32, kind="ExternalInput").ap(),
        "w_proj": nc.dram_tensor("w_proj", [D, D], BF16, kind="ExternalInput").ap(),
        "b_proj": nc.dram_tensor("b_proj", [D], F32, kind="ExternalInput").ap(),
        "tri": nc.dram_tensor("tri", [P, P], BF16, kind="ExternalInput").ap(),
        "bqv": nc.dram_tensor("bqv", [1, D], F32, kind="ExternalInput").ap(),
        "bqv16": nc.dram_tensor("bqv16", [1, D], BF16, kind="ExternalInput").ap(),
        "bp16": nc.dram_tensor("bp16", [1, D], BF16, kind="ExternalInput").ap(),
        "ident": nc.dram_tensor("ident", [P, P], BF16, kind="ExternalInput").ap(),
        "out": nc.dram_tensor("out", [T, D], F32, kind="ExternalOutput").ap(),
    }
    _build_tile_kernel(nc, aps)
    nc.compile()
    _CACHE["nc"] = nc
    return nc


def _host_consts():
    import ml_dtypes
    r = np.arange(P)
    tri = (r[:, None] <= r[None, :]).astype(ml_dtypes.bfloat16)
    ident = np.eye(P, dtype=ml_dtypes.bfloat16)
    return tri, ident


def kernel(x, w_qkv, b_qkv, w_proj, b_proj):
    x = np.ascontiguousarray(np.asarray(x, dtype=np.float32))
    w_qkv = np.ascontiguousarray(np.asarray(w_qkv, dtype=np.float32))
    b_qkv = np.ascontiguousarray(np.asarray(b_qkv, dtype=np.float32))
    w_proj = np.ascontiguousarray(np.asarray(w_proj, dtype=np.float32))
    b_proj = np.ascontiguousarray(np.asarray(b_proj, dtype=np.float32))

    nc = _get_nc()
    import ml_dtypes
    bf = ml_dtypes.bfloat16
    tri, ident = _host_consts()
    wq_bf = w_qkv.astype(bf)
    wp_bf = w_proj.astype(bf)
    bqv = np.ascontiguousarray(b_qkv[2048:3072].reshape(1, D))
    x_bf = x.astype(bf)
    in_maps = [
        {
            "x": x_bf[b],
            "w_qkv": wq_bf,
            "b_qkv": b_qkv,
            "w_proj": wp_bf,
            "b_proj": b_proj,
            "tri": tri,
            "bqv": bqv,
            "bqv16": bqv.astype(bf),
            "bp16": b_proj.reshape(1, D).astype(bf),
            "ident": ident,
        }
        for b in range(N_CORES)
    ]
    res = bass_utils.run_bass_kernel_spmd(
        nc, in_maps, core_ids=list(range(N_CORES)), trace=TRACE
    )
    LAST_RESULT["res"] = res
    return np.stack([res.results[c]["out"] for c in range(N_CORES)]).astype(
        np.float32
    )


# revision 21
# speedup vs baseline: 1.0028x; 1.0028x over previous
"""Causal self-attention Bass kernel for 8x Trainium2 NeuronCores.

Problem: B=8, T=1024, D=1024, H=16 heads (head_dim 64), fp32.
Sharding: data parallel over batch -- each of the 8 cores handles one
batch element with replicated weights; outputs are stacked on the host.

Per-core dataflow (all matmuls on PE in bf16 with fp32 PSUM accumulate;
weights are cast to bf16 on the host):
  1. x [T,D] is host-cast to bf16 (halves the startup DMA) and
     transposed on PE (128x128 blocks, 1 cycle/row at bf16) to xT [D,T];
     crossbar-transpose DMAs were tried and are ~8x too slow.
  2. v = x @ w_qkv[:,2048:] in natural layout with the bias added on DVE
     during the PSUM->SBUF cast against a stride-0-DMA bias tile; per
     head a ones column is interleaved at slot 0 so the AV matmul
     accumulates the softmax denominator at PSUM partition 0.
     qkT = (w_qkv[:, :2048])^T @ x^T kept transposed [2048,T]; q/k bias
     folded into the PSUM->SBUF cast as a per-partition DVE add.
     qk f-tiles are produced in chunk order (q03, k03, q47, k47) with
     head attention interleaved between the chunks so ACT/DVE softmax
     work overlaps the dense qk matmul stream on PE.
  3. Per head h: scoresT[tk, tq 0:1024] for both tq halves live in one
     two-bank PSUM tile, exp'd by a single bank-crossing ACT op per tk
     tile (scale=1/8 folded; no max-subtraction -- scores are O(1) so
     exp cannot overflow).  Causal handling computes only the exact
     unmasked column window (128-granular) plus one [128,128] triangular
     mask multiply per diagonal block on DVE.  o_aug[0:65, tq] +=
     v_aug^T @ P accumulates both halves in one two-bank PSUM tile.
     QK(i+1) is emitted before AV(i) so the exp chain never stalls PE.
  4. Normalization: one DVE reciprocal_approx_fast per head on the
     [1,1024] denominator row at PSUM partition 0, broadcast to 64
     partitions through a DRAM bounce with a stride-0 read (idle DMA
     bandwidth only), multiplied into the attention tile on DVE --
     deferred by one head so PE never waits on the chain.
  5. y = attn^T' @ w_proj with the bias added on DVE during the
     PSUM->SBUF copy, streamed back to DRAM.
"""

import numpy as np
from contextlib import ExitStack

import concourse.bass as bass
import concourse.bacc as bacc
import concourse.tile as tile
import concourse.mybir as mybir
from concourse import bass_utils

F32 = mybir.dt.float32
BF16 = mybir.dt.bfloat16
AF = mybir.ActivationFunctionType
OP = mybir.AluOpType

B, T, D, H, HD = 8, 1024, 1024, 16, 64
P = 128
N_CORES = 8

TRACE = False

_CACHE = {}
LAST_RESULT = {}


def _build_tile_kernel(nc, aps):
    x, wq, bq, wp, bp, tri, bqv, out = (
        aps["x"], aps["w_qkv"], aps["b_qkv"], aps["w_proj"], aps["b_proj"],
        aps["tri"], aps["bqv"], aps["out"],
    )

    with tile.TileContext(nc) as tc, ExitStack() as ctx:
        consts = ctx.enter_context(tc.tile_pool(name="consts", bufs=1))
        qk_pool = ctx.enter_context(tc.tile_pool(name="qk_pool", bufs=16))
        xt_pool = ctx.enter_context(tc.tile_pool(name="xt_pool", bufs=8))
        v_pool = ctx.enter_context(tc.tile_pool(name="v_pool", bufs=8))
        w_pool = ctx.enter_context(tc.tile_pool(name="w_pool", bufs=12))
        xn_pool = ctx.enter_context(tc.tile_pool(name="xn_pool", bufs=8))
        at_pool = ctx.enter_context(tc.tile_pool(name="at_pool", bufs=8))
        p_pool = ctx.enter_context(tc.tile_pool(name="p_pool", bufs=6))
        den_pool = ctx.enter_context(tc.tile_pool(name="den_pool", bufs=2))
        rb_pool = ctx.enter_context(tc.tile_pool(name="rb_pool", bufs=3))
        dr_pool = ctx.enter_context(
            tc.tile_pool(name="dr_pool", bufs=4, space="DRAM"))
        y_pool = ctx.enter_context(tc.tile_pool(name="y_pool", bufs=2))
        ps = ctx.enter_context(tc.tile_pool(name="ps", bufs=2, space="PSUM"))
        ops = ctx.enter_context(tc.tile_pool(name="ops", bufs=2, space="PSUM"))

        # ---- phase 1a: x -> xT (PE transpose of 128x128 blocks) --------
        # x and the identity go into the DMA queues FIRST -- everything at
        # startup waits on xT, and the 1MB of bias-broadcast DMAs would
        # otherwise delay it by ~10us.
        id_sb = consts.tile([P, P], BF16)
        nc.sync.dma_start(out=id_sb, in_=aps["ident"])
        xns = []
        for ti in range(8):
            xn = xn_pool.tile([P, D], BF16, name="xn", tag="xn")
            nc.sync.dma_start(out=xn, in_=x[ti * P:(ti + 1) * P, :])
            xns.append(xn)

        # ---- constants -------------------------------------------------
        tri_sb = consts.tile([P, P], BF16)
        nc.sync.dma_start(out=tri_sb, in_=tri)
        bcol_sb = consts.tile([P, 16], F32)  # b_qkv[0:2048] as per-partition cols
        nc.sync.dma_start(out=bcol_sb, in_=bq[0:2048].rearrange("(f p) -> p f", p=P))
        # bias rows replicated to all 128 partitions by stride-0 DMA (bf16
        # halves the broadcast bytes; the biases only feed bf16-adjacent
        # adds)
        bv_b = consts.tile([P, D], BF16)
        nc.sync.dma_start(out=bv_b, in_=aps["bqv16"].to_broadcast((P, D)))
        bp_b = consts.tile([P, D], BF16)
        nc.sync.dma_start(out=bp_b, in_=aps["bp16"].to_broadcast((P, D)))
        xt_tiles = [
            xt_pool.tile([P, T], BF16, name="xt", tag="xt") for _ in range(8)
        ]
        for jj in range(2):
            for r in range(4):
                ka, kb = r, r + 4
                pst = ps.tile([P, 1024], BF16, name="pst", tag="ps")
                for tt in range(4):
                    xn = xns[jj * 4 + tt]
                    nc.tensor.transpose(
                        pst[:, tt * P:(tt + 1) * P],
                        xn[:, ka * P:(ka + 1) * P], id_sb)
                    nc.tensor.transpose(
                        pst[:, 512 + tt * P:512 + (tt + 1) * P],
                        xn[:, kb * P:(kb + 1) * P], id_sb)
                nc.vector.tensor_copy(
                    xt_tiles[ka][:, jj * 512:(jj + 1) * 512], pst[:, 0:512])
                nc.vector.tensor_copy(
                    xt_tiles[kb][:, jj * 512:(jj + 1) * 512], pst[:, 512:1024])

        # ---- phase 1v: v natural layout ---------------------------------
        # v_tiles[m] is [128, 16*128] bf16: per head [ones | 63 pad | 64 v]
        # so the AV matmul puts the softmax denominator at PSUM partition 0
        # (readable by reciprocal_approx_fast) and o rows at partitions
        # 64..127 (64-partition DVE accesses must start at 0 or 64).
        v_tiles = []
        for m in range(8):
            vt = v_pool.tile([P, 16 * 128], BF16, name="vt", tag="v")
            rr0 = vt.rearrange("p (h c) -> p h c", c=128)
            nc.vector.memset(rr0[:, :, 0:1], 1.0)
            nc.vector.memset(rr0[:, :, 1:64], 0.0)
            v_tiles.append(vt)
        vw_tiles = {}
        for k in range(8):
            wt = w_pool.tile([P, 1024], BF16, name="wt", tag="w")
            nc.sync.dma_start(
                out=wt, in_=wq[k * P:(k + 1) * P, 2048:3072]
            )
            vw_tiles[k] = wt
        for m in range(8):
            acc = ps.tile([P, 1024], F32, name="acc", tag="ps")
            for k in range(8):
                xsl = xt_tiles[k][:, m * P:(m + 1) * P]
                nc.tensor.matmul(acc[:, 0:512], xsl, vw_tiles[k][:, 0:512],
                                 start=(k == 0), stop=(k == 7))
                nc.tensor.matmul(acc[:, 512:1024], xsl, vw_tiles[k][:, 512:1024],
                                 start=(k == 0), stop=(k == 7))
            rr = v_tiles[m].rearrange("p (h c) -> p h c", c=128)
            nc.vector.tensor_tensor(
                rr[:, :, 64:128], acc.rearrange("p (h c) -> p h c", c=64),
                bv_b.rearrange("p (h c) -> p h c", c=64), op=OP.add,
            )

        # ---- phase 1q + 2: qkT production interleaved with attention ---
        qk_tiles = {}  # f-tile index 0..15 -> [128, 1024] bf16

        def emit_qk_tile(f):
            # all 8 K-slices of w for this f-tile in one strided DMA
            wt = w_pool.tile([P, 8, P], BF16, name="wt", tag="w")
            nc.sync.dma_start(
                out=wt,
                in_=wq[:, f * P:(f + 1) * P].rearrange(
                    "(k p) f -> p k f", p=P),
            )
            qk_t = qk_pool.tile([P, T], BF16, name="qk_t", tag="qk")
            qk_tiles[f] = qk_t
            acc = ps.tile([P, 1024], F32, name="acc", tag="ps")
            for k in range(8):
                nc.tensor.matmul(acc[:, 0:512], wt[:, k, :],
                                 xt_tiles[k][:, 0:512],
                                 start=(k == 0), stop=(k == 7))
            # cast each half as soon as it completes so the PSUM ring slot
            # frees before the next head's scores matmuls need it
            nc.vector.tensor_scalar_add(
                qk_t[:, 0:512], acc[:, 0:512], bcol_sb[:, f:f + 1])
            for k in range(8):
                nc.tensor.matmul(acc[:, 512:1024], wt[:, k, :],
                                 xt_tiles[k][:, 512:1024],
                                 start=(k == 0), stop=(k == 7))
            nc.vector.tensor_scalar_add(
                qk_t[:, 512:1024], acc[:, 512:1024], bcol_sb[:, f:f + 1])

        att_tiles = {}  # hp -> [128, 1024] bf16 (tq halves side by side)
        for hp in range(8):
            att_tiles[hp] = at_pool.tile([P, T], BF16, name="at", tag="at")

        # deferred normalization: PE must never wait on the DVE recip ->
        # DMA bounce -> DVE mult chain, so each chain starts as soon as its
        # tq half finishes accumulating and the multiply lands one head
        # later (pn_early: first halves; pn_late: second halves).
        pn_early = []
        pn_late = []

        def start_norm(o_ps, att_slice, lo, hi, queue):
            dinv = den_pool.tile([1, hi - lo], F32, name="dinv", tag="dinv")
            nc.vector.reciprocal_approx_fast(dinv, o_ps[0:1, lo:hi])
            dd = dr_pool.tile([1, hi - lo], F32, name="dd", tag="dd")
            nc.sync.dma_start(out=dd, in_=dinv)
            rb_sb = rb_pool.tile([64, hi - lo], F32, name="rb", tag="rb")
            nc.sync.dma_start(out=rb_sb, in_=dd.to_broadcast((64, hi - lo)))
            queue.append((rb_sb, o_ps, lo, hi, att_slice))

        def flush_norm(queue):
            while queue:
                rb_sb, o_ps, lo, hi, att_slice = queue.pop(0)
                nc.vector.tensor_tensor(
                    att_slice[:, lo:hi], o_ps[64:128, lo:hi], rb_sb,
                    op=OP.mult)

        def emit_head(h):
            """Attention for head h, both tq halves fused: scores tile i
            covers tq cols [128i..512) (first half, causal window) and
            [512..1024) (second half) in one 2-bank PSUM tile with a
            single bank-crossing exp."""
            fq = h // 2
            po = (h % 2) * 64
            qT = qk_tiles[fq][po:po + 64, :]
            kTt = qk_tiles[8 + fq]
            o_ps = ops.tile([P, 1024], F32, name="o_ps", tag="ops")
            att_slice = att_tiles[h // 2][(h % 2) * 64:(h % 2) * 64 + 64, :]
            pending = None

            def emit_av(pend, stop):
                pi, pp = pend
                va = v_tiles[pi].rearrange("p (h c) -> p h c", c=128)[:, h, :]
                if pi < 4:  # first-half window [128*pi .. 512)
                    nc.tensor.matmul(
                        o_ps[:, P * pi:512], va, pp[:, P * pi:512],
                        start=(pi == 0), stop=(pi == 3),
                        skip_group_check=True,
                    )
                    nc.tensor.matmul(
                        o_ps[:, 512:1024], va, pp[:, 512:1024],
                        start=(pi == 0), stop=stop,
                        skip_group_check=True,
                    )
                else:  # second half only, window [512+128m .. 1024)
                    ws = 512 + P * (pi - 4)
                    nc.tensor.matmul(
                        o_ps[:, ws:], va, pp[:, ws:],
                        start=False, stop=stop,
                        skip_group_check=True,
                    )

            for i in range(8):
                kT = kTt[po:po + 64, i * P:(i + 1) * P]
                s_ps = ps.tile([P, 1024], F32, name="s_ps", tag="ps")
                p_sb = p_pool.tile([P, 1024], BF16, name="p_sb", tag="p")
                if i < 4:
                    ws = P * i
                    nc.tensor.matmul(s_ps[:, ws:512], kT, qT[:, ws:512],
                                     start=True, stop=True)
                    nc.tensor.matmul(s_ps[:, 512:1024], kT, qT[:, 512:1024],
                                     start=True, stop=True)
                    nc.scalar.activation(
                        p_sb[:, ws:], s_ps[:, ws:], AF.Exp, scale=0.125)
                else:
                    ws = 512 + P * (i - 4)
                    nc.tensor.matmul(s_ps[:, ws:], kT, qT[:, ws:],
                                     start=True, stop=True)
                    nc.scalar.activation(
                        p_sb[:, ws:], s_ps[:, ws:], AF.Exp, scale=0.125)
                nc.vector.tensor_tensor(
                    p_sb[:, ws:ws + P], p_sb[:, ws:ws + P],
                    tri_sb, op=OP.mult)
                if i == 2:
                    flush_norm(pn_early)
                if pending is not None:
                    emit_av(pending, stop=False)
                pending = (i, p_sb)
            emit_av(pending, stop=True)
            start_norm(o_ps, att_slice, 0, 1024, pn_early)

        # Head pair (2c, 2c+1) needs q f-tile c and k f-tile 8+c.  Emit
        # each f-tile production (a dense 8.2k-cycle PE chunk) right before
        # the head pair that consumes it, so the ACT/DVE softmax work of
        # every pair overlaps a dense qk matmul stream on PE.
        emit_qk_tile(0)
        emit_qk_tile(8)
        emit_qk_tile(1)
        emit_qk_tile(9)
        emit_qk_tile(2)
        wp_tiles = {}

        def load_wp():
            for c in range(8):
                wpt = w_pool.tile([P, 1024], BF16, name="wpt", tag="w")
                nc.sync.dma_start(out=wpt, in_=wp[c * P:(c + 1) * P, :])
                wp_tiles[c] = wpt

        for c in range(8):
            emit_head(2 * c)
            if c < 6:
                emit_qk_tile(10 + c)
            emit_head(2 * c + 1)
            if c < 5:
                emit_qk_tile(c + 3)
            if c == 5:
                load_wp()  # prefetch proj weights on idle DMA
        flush_norm(pn_early)

        # ---- phase 3: projection ---------------------------------------
        for j in range(2):
            for mi in range(4):
                mrow = 4 * j + mi
                y_ps = ps.tile([P, 1024], F32, name="y_ps", tag="ps")
                for c in range(8):
                    asl = att_tiles[c][:, j * 512 + mi * P:j * 512 + (mi + 1) * P]
                    nc.tensor.matmul(y_ps[:, 0:512], asl, wp_tiles[c][:, 0:512],
                                     start=(c == 0), stop=(c == 7))
                    nc.tensor.matmul(y_ps[:, 512:1024], asl,
                                     wp_tiles[c][:, 512:1024],
                                     start=(c == 0), stop=(c == 7))
                y_sb = y_pool.tile([P, 1024], F32, name="y_sb", tag="y")
                nc.vector.tensor_tensor(y_sb, y_ps, bp_b, op=OP.add)
                nc.sync.dma_start(
                    out=out[mrow * P:(mrow + 1) * P, :], in_=y_sb)


def _pin_act_table(arch):
    """Force every ACT func we use into one table so walrus never emits
    mid-kernel ACT_TABLE_LOADs (each is ~1.3us on the ScalarE stream).
    The cached dict is mutated in place, preserving set ids/order."""
    import concourse.hw_specs as hw_specs
    tabs = hw_specs.get_activation_tables(arch)
    keep = "natural_log_exp_and_others"
    if keep not in tabs:
        return
    need = tabs[keep] & {AF.Exp, AF.Ln, AF.Copy, AF.Identity}
    for name, fns in tabs.items():
        if name != keep:
            fns -= need


def _get_nc():
    if "nc" in _CACHE:
        return _CACHE["nc"]
    nc = bacc.Bacc("TRN2", target_bir_lowering=False, debug=False,
                   num_devices=N_CORES)
    _pin_act_table(nc.m.arch)
    aps = {
        "x": nc.dram_tensor("x", [T, D], BF16, kind="ExternalInput").ap(),
        "w_qkv": nc.dram_tensor("w_qkv", [D, 3 * D], BF16, kind="ExternalInput").ap(),
        "b_qkv": nc.dram_tensor("b_qkv", [3 * D], F32, kind="ExternalInput").ap(),
        "w_proj": nc.dram_tensor("w_proj", [D, D], BF16, kind="ExternalInput").ap(),
        "b_proj": nc.dram_tensor("b_proj", [D], F32, kind="ExternalInput").ap(),
        "tri": nc.dram_tensor("tri", [P, P], BF16, kind="ExternalInput").ap(),
        "bqv": nc.dram_tensor("bqv", [1, D], F32, kind="ExternalInput").ap(),
        "bqv16": nc.dram_tensor("bqv16", [1, D], BF16, kind="ExternalInput").ap(),
        "bp16": nc.dram_tensor("bp16", [1, D], BF16, kind="ExternalInput").ap(),
        "ident": nc.dram_tensor("ident", [P, P], BF16, kind="ExternalInput").ap(),
        "out": nc.dram_tensor("out", [T, D], F32, kind="ExternalOutput").ap(),
    }
    _build_tile_kernel(nc, aps)
    nc.compile()
    _CACHE["nc"] = nc
    return nc


def _host_consts():
    import ml_dtypes
    r = np.arange(P)
    tri = (r[:, None] <= r[None, :]).astype(ml_dtypes.bfloat16)
    ident = np.eye(P, dtype=ml_dtypes.bfloat16)
    return tri, ident


def kernel(x, w_qkv, b_qkv, w_proj, b_proj):
    x = np.ascontiguousarray(np.asarray(x, dtype=np.float32))
    w_qkv = np.ascontiguousarray(np.asarray(w_qkv, dtype=np.float32))
    b_qkv = np.ascontiguousarray(np.asarray(b_qkv, dtype=np.float32))
    w_proj = np.ascontiguousarray(np.asarray(w_proj, dtype=np.float32))
    b_proj = np.ascontiguousarray(np.asarray(b_proj, dtype=np.float32))

    nc = _get_nc()
    import ml_dtypes
    bf = ml_dtypes.bfloat16
    tri, ident = _host_consts()
    wq_bf = w_qkv.astype(bf)
    wp_bf = w_proj.astype(bf)
    bqv = np.ascontiguousarray(b_qkv[2048:3072].reshape(1, D))
    x_bf = x.astype(bf)
    in_maps = [
        {
            "x": x_bf[b],
            "w_qkv": wq_bf,
            "b_qkv": b_qkv,
            "w_proj": wp_bf,
            "b_proj": b_proj,
            "tri": tri,
            "bqv": bqv,
            "bqv16": bqv.astype(bf),
            "bp16": b_proj.reshape(1, D).astype(bf),
            "ident": ident,
        }
        for b in range(N_CORES)
    ]
    res = bass_utils.run_bass_kernel_spmd(
        nc, in_maps, core_ids=list(range(N_CORES)), trace=TRACE
    )
    LAST_RESULT["res"] = res
    return np.stack([res.results[c]["out"] for c in range(N_CORES)]).astype(
        np.float32
    )
